# revision 13
# baseline (speedup 1.0000x reference)
"""Trainium2 Bass kernel for the ContinuousLS column-selection module.

Strategy
--------
The reference does:
  1. residual col norms of A after projecting out span(S)  -> sampling logits
  2. Gumbel top-(10k) candidate set C (RNG key 42 => input-independent noise)
  3. selected set sel_idx via norm-matching S's columns against A's columns
  4. K = A^T A, K2 = K @ K, then 640 pair objectives
     val(p,q) = ||A||_F^2 - tr(pinv(G) M) over 9x9 masked submatrices of
     K / K2 at indices [sel_idx, p]
  5. argmin -> swap one column; output A[:, out_idx]

Key algebraic reduction: the pair objectives only touch K and K2 at the
88 indices B = sel_idx (8) + C (80).  With Z = A[:, B]^T A  ([88, 1024]):
    K[B, B]  = Z[:, B]
    K2[B, B] = Z @ Z^T
so the only large computation needed is Z (1.5 GFLOP, one full read of A)
instead of K (17 GFLOP) and K2 (2 GFLOP).  Z is computed on the 8
NeuronCores, row-sharded over A's 8192 rows (contraction dim) with
per-core partial sums reduced on the host.

The discrete decisions (which columns match S by norm, Gumbel ranking)
have razor-thin margins (measured 7.8e-7 on the norm-match threshold), so
the norm computations and RNG draws are replicated bitwise with the same
jax-on-CPU ops the reference uses.  All remaining margins are >= 4e-3,
far above f32 GEMM noise.
"""

import numpy as np

EPS = 1e-10

_CACHE = {}


# ----------------------------------------------------------------- device ---

def _build_z_kernel(n_rows_per_core, d, nB, n_cores):
    """Bass program: per core, Z_partial = A_B_shard^T @ A_shard.

    A_shard   [n_rows_per_core, d]   (rows = contraction dim)
    AB_shard  [n_rows_per_core, nB]
    Z_partial [nB, d]
    """
    import concourse.mybir as mybir
    import concourse.tile as tile
    from concourse import bacc

    P = 128
    assert n_rows_per_core % P == 0
    n_chunks = n_rows_per_core // P           # 8 for 1024 rows/core
    NT = 512                                  # fp32 moving-operand max
    assert d % NT == 0
    n_ntiles = d // NT                        # 2 for d=1024

    nc = bacc.Bacc("TRN2", target_bir_lowering=False, debug=False,
                   num_devices=n_cores)
    a_in = nc.dram_tensor("a_shard", [n_rows_per_core, d],
                          mybir.dt.float32, kind="ExternalInput")
    # ab_shard is pre-swizzled on the host into SBUF layout:
    # ab[p, t*nB + b] = AB[t*128 + p, b]  (contiguous per partition)
    ab_in = nc.dram_tensor("ab_shard", [P, n_chunks * nB],
                           mybir.dt.float32, kind="ExternalInput")
    z_out = nc.dram_tensor("z_partial", [nB, d],
                           mybir.dt.float32, kind="ExternalOutput")

    # row t*128+p lives at partition p
    a_view = a_in.rearrange("(t p) m -> p t m", p=P)

    khalf = n_chunks // 2

    with tile.TileContext(nc) as tc:
        with tc.tile_pool(name="achunk", bufs=n_chunks) as apool, \
             tc.tile_pool(name="ab", bufs=1) as abpool, \
             tc.tile_pool(name="zout", bufs=n_ntiles) as zpool, \
             tc.tile_pool(name="psum", bufs=1, space="PSUM") as psum:
            # ab on the ACT HWDGE ring so it doesn't delay A streaming on SP
            ab_sb = abpool.tile([P, n_chunks * nB], mybir.dt.float32)
            nc.scalar.dma_start(ab_sb[:], ab_in[:])

            pts = [psum.tile([nB, NT], mybir.dt.float32, name=f"pt{h}")
                   for h in range(n_ntiles)]
            a_tiles = []
            for t in range(n_chunks):
                a_sb = apool.tile([P, d], mybir.dt.float32, name="a_sb",
                                  tag="achunk")
                if t == 0:
                    # split the first chunk so the PE can start sooner
                    for h in range(n_ntiles):
                        nc.sync.dma_start(a_sb[:, h * NT:(h + 1) * NT],
                                          a_view[:, t, h * NT:(h + 1) * NT])
                else:
                    nc.sync.dma_start(a_sb[:], a_view[:, t, :])
                a_tiles.append(a_sb)
            for t in range(n_chunks):
                lhsT = ab_sb[:, t * nB:(t + 1) * nB]
                for h in range(n_ntiles):
                    nc.tensor.matmul(
                        pts[h][:], lhsT,
                        a_tiles[t][:, h * NT:(h + 1) * NT],
                        start=(t == 0), stop=(t == n_chunks - 1))
            # pt[0]'s last matmul lands one mm before pt[1]'s, so its
            # PSUM->SBUF copy overlaps the final matmul.
            for h in range(n_ntiles):
                z_sb = zpool.tile([nB, NT], mybir.dt.float32, name="z_sb",
                                  tag="zout")
                nc.vector.tensor_copy(z_sb[:], pts[h][:])
                nc.scalar.dma_start(z_out[:, h * NT:(h + 1) * NT], z_sb[:])
    nc.compile()
    return nc


def _run_z(A, AB, n_cores=8):
    """Compute Z = AB^T @ A on the 8 NeuronCores (row-sharded)."""
    from concourse.bass_utils import run_bass_kernel_spmd

    n, d = A.shape
    nB = AB.shape[1]
    rows_per_core = n // n_cores
    key = (rows_per_core, d, nB, n_cores)
    if key not in _CACHE:
        _CACHE[key] = _build_z_kernel(rows_per_core, d, nB, n_cores)
    nc = _CACHE[key]

    # pre-swizzle AB into the kernel's SBUF layout:
    # [n, nB] -> per core [128, n_chunks*nB] with ab[p, t*nB+b] = AB[t*128+p, b]
    n_chunks = rows_per_core // 128
    AB_sw = np.ascontiguousarray(
        AB.reshape(n_cores, n_chunks, 128, nB)
        .transpose(0, 2, 1, 3)
        .reshape(n_cores, 128, n_chunks * nB))
    in_maps = []
    for c in range(n_cores):
        sl = slice(c * rows_per_core, (c + 1) * rows_per_core)
        in_maps.append({
            "a_shard": np.ascontiguousarray(A[sl]),
            "ab_shard": AB_sw[c],
        })
    res = run_bass_kernel_spmd(nc, in_maps, list(range(n_cores)))
    parts = np.stack([res.results[c]["z_partial"] for c in range(n_cores)])
    return parts.astype(np.float64).sum(axis=0)


# ------------------------------------------------------------------- host ---

def _host_reference_bits(A, S, num_samples):
    """The pieces that must match the reference bit-for-bit: f32 column
    norms (the 1e-5 match threshold has ~1e-6 margins) and the RNG draws
    (input-independent, key 42)."""
    import jax
    import jax.numpy as jnp

    cpu = jax.devices("cpu")[0]
    with jax.default_device(cpu):
        a_norms = np.asarray(jnp.linalg.norm(jnp.asarray(A), axis=0))
        s_norms = np.asarray(jnp.linalg.norm(jnp.asarray(S), axis=0))
        kg, km = jax.random.split(jax.random.key(42))
        u = np.asarray(jax.random.uniform(kg, (A.shape[1],),
                                          dtype=jnp.float32))
        rand_idx = int(np.asarray(
            jax.random.randint(km, (), 0, num_samples)))
    return a_norms, s_norms, u, rand_idx


def _topk_desc_stable(values, k):
    """jax.lax.top_k semantics: k largest, ties -> lower index first."""
    order = np.argsort(-values, kind="stable")
    return order[:k]


def _pinv_jaxlike(mats):
    """Batched pseudo-inverse with jax's f32 pinv rank cutoff
    (rtol = max(M,N) * eps_f32 relative to the largest singular value)."""
    u, s, vh = np.linalg.svd(mats)
    cutoff = (mats.shape[-1] * np.finfo(np.float32).eps
              * s[..., :1])
    s_inv = np.where(s > cutoff, 1.0 / np.where(s > 0, s, 1.0), 0.0)
    return np.einsum("...ji,...j,...kj->...ik", vh, s_inv, u)


def kernel(A_prime, k, S):
    A = np.ascontiguousarray(np.asarray(A_prime, dtype=np.float32))
    S = np.ascontiguousarray(np.asarray(S, dtype=np.float32))
    kk = int(np.asarray(k))
    n, d = A.shape
    s = S.shape[1]
    num_samples = min(10 * kk, d)

    a_norms, s_norms, u, rand_idx = _host_reference_bits(A, S, num_samples)

    # I_soft: columns of A matching a column of S by relative norm
    a64 = a_norms.astype(np.float64)
    s64 = s_norms.astype(np.float64)
    match = (np.abs(s64[None, :] - a64[:, None])
             / (a64[:, None] + EPS)) < 1e-5
    I_soft = match.any(axis=1).astype(np.float32)
    sel_idx = np.sort(_topk_desc_stable(I_soft, s))

    # G_S and the projection weights (small, host f64; margins ~7e-3)
    S64 = S.astype(np.float64)
    G_S = S64.T @ S64
    T = S64.T @ A.astype(np.float64)                  # [s, d]
    W = np.linalg.pinv(G_S) @ T
    a2 = a64 * a64
    col_norms = np.maximum(a2 - np.einsum("sd,sd->d", T, W), 0.0)

    probs = col_norms / (col_norms.sum() + EPS)
    gumbel = -np.log(-np.log(u.astype(np.float64) + EPS) + EPS)
    logits = np.log(probs + EPS) + gumbel
    C_indices = _topk_desc_stable(logits, num_samples)

    # --- device: Z = A[:, B]^T A, row-sharded over the 8 cores ---
    B = np.concatenate([sel_idx, C_indices]).astype(np.int64)
    AB = np.ascontiguousarray(A[:, B])
    Z = _run_z(A, AB)                                  # [s+ns, d] float64

    Ksub = Z[:, B]                                     # K[B, B]
    K2sub = Z @ Z.T                                    # K2[B, B]
    A_fro2 = float(a2.sum())

    # --- 640 pair objectives (tiny, host f64) ---
    ns = num_samples
    sel_pos = np.arange(s)
    # G/M for each candidate p: rows/cols [0..s-1] = sel, row/col s = p
    idx9 = np.empty((ns, s + 1), np.int64)
    idx9[:, :s] = np.arange(s)[None, :]
    idx9[:, s] = s + np.arange(ns)
    Gall = Ksub[idx9[:, :, None], idx9[:, None, :]]    # [ns, 9, 9]
    Mall = K2sub[idx9[:, :, None], idx9[:, None, :]]
    # masks: [ns, s, 9]: remove qpos; if p == sel[q], remove p too
    mask = np.ones((ns, s, s + 1))
    mask[:, sel_pos, sel_pos] = 0.0
    p_eq_q = (C_indices[:, None] == sel_idx[None, :])  # [ns, s]
    mask[:, :, s] = np.where(p_eq_q, 0.0, 1.0)
    mm = mask[:, :, :, None] * mask[:, :, None, :]     # [ns, s, 9, 9]
    Gm = mm * Gall[:, None]
    Mm = mm * Mall[:, None]
    pinvs = _pinv_jaxlike(Gm.reshape(-1, s + 1, s + 1))
    tr = np.einsum("bij,bij->b", pinvs,
                   Mm.reshape(-1, s + 1, s + 1))
    objs = np.sqrt(np.maximum(A_fro2 - tr, 0.0)).reshape(ns, s)

    amin = int(np.argmin(objs.reshape(-1)))
    min_idx = int(sel_idx[amin % s])
    best_p_idx = int(C_indices[rand_idx])

    I_final = I_soft.copy()
    I_final[min_idx] = 0.0
    I_final[best_p_idx] = 1.0
    out_idx = np.sort(_topk_desc_stable(I_final, s))
    return np.ascontiguousarray(A[:, out_idx])


# revision 15
# speedup vs baseline: 34636.7876x; 34636.7876x over previous
"""Trainium2 Bass kernel for the ContinuousLS column-selection module.

Strategy
--------
The reference does:
  1. residual col norms of A after projecting out span(S)  -> sampling logits
  2. Gumbel top-(10k) candidate set C (RNG key 42 => input-independent noise)
  3. selected set sel_idx via norm-matching S's columns against A's columns
  4. K = A^T A, K2 = K @ K, then 640 pair objectives
     val(p,q) = ||A||_F^2 - tr(pinv(G) M) over 9x9 masked submatrices of
     K / K2 at indices [sel_idx, p]
  5. argmin -> swap one column; output A[:, out_idx]

Key algebraic reduction: the pair objectives only touch K and K2 at the
88 indices B = sel_idx (8) + C (80).  With Z = A[:, B]^T A  ([88, 1024]):
    K[B, B]  = Z[:, B]
    K2[B, B] = Z @ Z^T
so the only large computation needed is Z (1.5 GFLOP, one full read of A)
instead of K (17 GFLOP) and K2 (2 GFLOP).  Z is computed on the 8
NeuronCores, row-sharded over A's 8192 rows (contraction dim) with
per-core partial sums reduced on the host.

The discrete decisions (which columns match S by norm, Gumbel ranking)
have razor-thin margins (measured 7.8e-7 on the norm-match threshold), so
the norm computations and RNG draws are replicated bitwise with the same
jax-on-CPU ops the reference uses.  All remaining margins are >= 4e-3,
far above f32 GEMM noise.
"""

import numpy as np

EPS = 1e-10

_CACHE = {}


# ----------------------------------------------------------------- device ---

def _build_z_kernel(n_rows_per_core, d, nB, n_cores, repeat=1):
    """Bass program: per core, Z_partial = A_B_shard^T @ A_shard.

    A_shard   [n_rows_per_core, d]   (rows = contraction dim)
    AB_shard  [n_rows_per_core, nB]  (pre-swizzled, see _run_z)
    Z_partial [nB, d]

    repeat > 1 wraps the body in a hardware loop; only used by the test
    harness to measure per-iteration device time by differencing.
    """
    import concourse.mybir as mybir
    import concourse.tile as tile
    from concourse import bacc

    P = 128
    assert n_rows_per_core % P == 0
    n_chunks = n_rows_per_core // P           # 8 for 1024 rows/core
    NT = 512                                  # fp32 moving-operand max
    assert d % NT == 0
    n_ntiles = d // NT                        # 2 for d=1024

    nc = bacc.Bacc("TRN2", target_bir_lowering=False, debug=False,
                   num_devices=n_cores)
    a_in = nc.dram_tensor("a_shard", [n_rows_per_core, d],
                          mybir.dt.float32, kind="ExternalInput")
    # ab_shard is pre-swizzled on the host into SBUF layout:
    # ab[p, t*nB + b] = AB[t*128 + p, b]  (contiguous per partition)
    ab_in = nc.dram_tensor("ab_shard", [P, n_chunks * nB],
                           mybir.dt.float32, kind="ExternalInput")
    z_out = nc.dram_tensor("z_partial", [nB, d],
                           mybir.dt.float32, kind="ExternalOutput")

    # row t*128+p lives at partition p
    a_view = a_in.rearrange("(t p) m -> p t m", p=P)

    with tile.TileContext(nc) as tc:
        with tc.tile_pool(name="achunk", bufs=n_chunks) as apool, \
             tc.tile_pool(name="ab", bufs=1) as abpool, \
             tc.tile_pool(name="zout", bufs=n_ntiles) as zpool, \
             tc.tile_pool(name="psum", bufs=1, space="PSUM") as psum:

            def body(_i=None):
                # ab on the ACT HWDGE ring so it doesn't delay A's stream
                ab_sb = abpool.tile([P, n_chunks * nB], mybir.dt.float32,
                                    name="ab_sb", tag="ab")
                nc.scalar.dma_start(ab_sb[:], ab_in[:])

                pts = [psum.tile([nB, NT], mybir.dt.float32, name=f"pt{h}",
                                 tag=f"pt{h}")
                       for h in range(n_ntiles)]
                a_tiles = []
                for t in range(n_chunks):
                    a_sb = apool.tile([P, d], mybir.dt.float32, name="a_sb",
                                      tag="achunk")
                    if t == 0:
                        # split the first chunk so the PE can start sooner
                        for h in range(n_ntiles):
                            nc.sync.dma_start(
                                a_sb[:, h * NT:(h + 1) * NT],
                                a_view[:, t, h * NT:(h + 1) * NT])
                    else:
                        nc.sync.dma_start(a_sb[:], a_view[:, t, :])
                    a_tiles.append(a_sb)
                for t in range(n_chunks):
                    lhsT = ab_sb[:, t * nB:(t + 1) * nB]
                    for h in range(n_ntiles):
                        nc.tensor.matmul(
                            pts[h][:], lhsT,
                            a_tiles[t][:, h * NT:(h + 1) * NT],
                            start=(t == 0), stop=(t == n_chunks - 1))
                # pt[0]'s last matmul lands one mm before pt[1]'s, so its
                # PSUM->SBUF copy overlaps the final matmul.
                for h in range(n_ntiles):
                    z_sb = zpool.tile([nB, NT], mybir.dt.float32,
                                      name="z_sb", tag="zout")
                    nc.vector.tensor_copy(z_sb[:], pts[h][:])
                    nc.scalar.dma_start(z_out[:, h * NT:(h + 1) * NT],
                                        z_sb[:])

            if repeat == 1:
                body()
            else:
                with tc.For_i(0, repeat, 1) as i:
                    body(i)
    nc.compile()
    return nc


def _run_z(A, AB, n_cores=8):
    """Compute Z = AB^T @ A on the 8 NeuronCores (row-sharded)."""
    from concourse.bass_utils import run_bass_kernel_spmd

    n, d = A.shape
    nB = AB.shape[1]
    rows_per_core = n // n_cores
    key = (rows_per_core, d, nB, n_cores)
    if key not in _CACHE:
        _CACHE[key] = _build_z_kernel(rows_per_core, d, nB, n_cores)
    nc = _CACHE[key]

    # pre-swizzle AB into the kernel's SBUF layout:
    # [n, nB] -> per core [128, n_chunks*nB] with ab[p, t*nB+b] = AB[t*128+p, b]
    n_chunks = rows_per_core // 128
    AB_sw = np.ascontiguousarray(
        AB.reshape(n_cores, n_chunks, 128, nB)
        .transpose(0, 2, 1, 3)
        .reshape(n_cores, 128, n_chunks * nB))
    in_maps = []
    for c in range(n_cores):
        sl = slice(c * rows_per_core, (c + 1) * rows_per_core)
        in_maps.append({
            "a_shard": np.ascontiguousarray(A[sl]),
            "ab_shard": AB_sw[c],
        })
    res = run_bass_kernel_spmd(nc, in_maps, list(range(n_cores)))
    parts = np.stack([res.results[c]["z_partial"] for c in range(n_cores)])
    return parts.astype(np.float64).sum(axis=0)


# ------------------------------------------------------------------- host ---

def _host_reference_bits(A, S, num_samples):
    """The pieces that must match the reference bit-for-bit: f32 column
    norms (the 1e-5 match threshold has ~1e-6 margins) and the RNG draws
    (input-independent, key 42)."""
    import jax
    import jax.numpy as jnp

    cpu = jax.devices("cpu")[0]
    with jax.default_device(cpu):
        a_norms = np.asarray(jnp.linalg.norm(jnp.asarray(A), axis=0))
        s_norms = np.asarray(jnp.linalg.norm(jnp.asarray(S), axis=0))
        kg, km = jax.random.split(jax.random.key(42))
        u = np.asarray(jax.random.uniform(kg, (A.shape[1],),
                                          dtype=jnp.float32))
        rand_idx = int(np.asarray(
            jax.random.randint(km, (), 0, num_samples)))
    return a_norms, s_norms, u, rand_idx


def _topk_desc_stable(values, k):
    """jax.lax.top_k semantics: k largest, ties -> lower index first."""
    order = np.argsort(-values, kind="stable")
    return order[:k]


def _pinv_jaxlike(mats):
    """Batched pseudo-inverse with jax's f32 pinv rank cutoff
    (rtol = max(M,N) * eps_f32 relative to the largest singular value)."""
    u, s, vh = np.linalg.svd(mats)
    cutoff = (mats.shape[-1] * np.finfo(np.float32).eps
              * s[..., :1])
    s_inv = np.where(s > cutoff, 1.0 / np.where(s > 0, s, 1.0), 0.0)
    return np.einsum("...ji,...j,...kj->...ik", vh, s_inv, u)


def kernel(A_prime, k, S):
    A = np.ascontiguousarray(np.asarray(A_prime, dtype=np.float32))
    S = np.ascontiguousarray(np.asarray(S, dtype=np.float32))
    kk = int(np.asarray(k))
    n, d = A.shape
    s = S.shape[1]
    num_samples = min(10 * kk, d)

    a_norms, s_norms, u, rand_idx = _host_reference_bits(A, S, num_samples)

    # I_soft: columns of A matching a column of S by relative norm
    a64 = a_norms.astype(np.float64)
    s64 = s_norms.astype(np.float64)
    match = (np.abs(s64[None, :] - a64[:, None])
             / (a64[:, None] + EPS)) < 1e-5
    I_soft = match.any(axis=1).astype(np.float32)
    sel_idx = np.sort(_topk_desc_stable(I_soft, s))

    # G_S and the projection weights (small, host f64; margins ~7e-3)
    S64 = S.astype(np.float64)
    G_S = S64.T @ S64
    T = S64.T @ A.astype(np.float64)                  # [s, d]
    W = np.linalg.pinv(G_S) @ T
    a2 = a64 * a64
    col_norms = np.maximum(a2 - np.einsum("sd,sd->d", T, W), 0.0)

    probs = col_norms / (col_norms.sum() + EPS)
    gumbel = -np.log(-np.log(u.astype(np.float64) + EPS) + EPS)
    logits = np.log(probs + EPS) + gumbel
    C_indices = _topk_desc_stable(logits, num_samples)

    # --- device: Z = A[:, B]^T A, row-sharded over the 8 cores ---
    B = np.concatenate([sel_idx, C_indices]).astype(np.int64)
    AB = np.ascontiguousarray(A[:, B])
    Z = _run_z(A, AB)                                  # [s+ns, d] float64

    Ksub = Z[:, B]                                     # K[B, B]
    K2sub = Z @ Z.T                                    # K2[B, B]
    A_fro2 = float(a2.sum())

    # --- 640 pair objectives (tiny, host f64) ---
    ns = num_samples
    sel_pos = np.arange(s)
    # G/M for each candidate p: rows/cols [0..s-1] = sel, row/col s = p
    idx9 = np.empty((ns, s + 1), np.int64)
    idx9[:, :s] = np.arange(s)[None, :]
    idx9[:, s] = s + np.arange(ns)
    Gall = Ksub[idx9[:, :, None], idx9[:, None, :]]    # [ns, 9, 9]
    Mall = K2sub[idx9[:, :, None], idx9[:, None, :]]
    # masks: [ns, s, 9]: remove qpos; if p == sel[q], remove p too
    mask = np.ones((ns, s, s + 1))
    mask[:, sel_pos, sel_pos] = 0.0
    p_eq_q = (C_indices[:, None] == sel_idx[None, :])  # [ns, s]
    mask[:, :, s] = np.where(p_eq_q, 0.0, 1.0)
    mm = mask[:, :, :, None] * mask[:, :, None, :]     # [ns, s, 9, 9]
    Gm = mm * Gall[:, None]
    Mm = mm * Mall[:, None]
    pinvs = _pinv_jaxlike(Gm.reshape(-1, s + 1, s + 1))
    tr = np.einsum("bij,bij->b", pinvs,
                   Mm.reshape(-1, s + 1, s + 1))
    objs = np.sqrt(np.maximum(A_fro2 - tr, 0.0)).reshape(ns, s)

    amin = int(np.argmin(objs.reshape(-1)))
    min_idx = int(sel_idx[amin % s])
    best_p_idx = int(C_indices[rand_idx])

    I_final = I_soft.copy()
    I_final[min_idx] = 0.0
    I_final[best_p_idx] = 1.0
    out_idx = np.sort(_topk_desc_stable(I_final, s))
    return np.ascontiguousarray(A[:, out_idx])


# revision 17
# speedup vs baseline: 36541.6877x; 1.0550x over previous
"""Trainium2 Bass kernel for the ContinuousLS column-selection module.

Strategy
--------
The reference does:
  1. residual col norms of A after projecting out span(S)  -> sampling logits
  2. Gumbel top-(10k) candidate set C (RNG key 42 => input-independent noise)
  3. selected set sel_idx via norm-matching S's columns against A's columns
  4. K = A^T A, K2 = K @ K, then 640 pair objectives
     val(p,q) = ||A||_F^2 - tr(pinv(G) M) over 9x9 masked submatrices of
     K / K2 at indices [sel_idx, p]
  5. argmin -> swap one column; output A[:, out_idx]

Key algebraic reduction: the pair objectives only touch K and K2 at the
88 indices B = sel_idx (8) + C (80).  With Z = A[:, B]^T A  ([88, 1024]):
    K[B, B]  = Z[:, B]
    K2[B, B] = Z @ Z^T
so the only large computation needed is Z (1.5 GFLOP, one full read of A)
instead of K (17 GFLOP) and K2 (2 GFLOP).  Z is computed on the 8
NeuronCores, row-sharded over A's 8192 rows (contraction dim) with
per-core partial sums reduced on the host.

The discrete decisions (which columns match S by norm, Gumbel ranking)
have razor-thin margins (measured 7.8e-7 on the norm-match threshold), so
the norm computations and RNG draws are replicated bitwise with the same
jax-on-CPU ops the reference uses.  All remaining margins are >= 4e-3,
far above f32 GEMM noise.
"""

import numpy as np

EPS = 1e-10

_CACHE = {}


# ----------------------------------------------------------------- device ---

def _build_z_kernel(n_rows_per_core, d, nB, n_cores, repeat=1):
    """Bass program: per core, Z_partial = A_B_shard^T @ A_shard.

    A is shipped as an f16 hi/lo pair (hi = f16(A), lo = f16(A - hi)) and
    the product expanded as hi*hi + hi*lo + lo*hi (the lo*lo term is
    ~2^-22 relative - measured objective error 4e-6 vs 4e-3 margins, same
    class as a plain f32 matmul).  f16 matmuls run at 1 cycle/row vs 4
    for f32, so this cuts PE time 25% for identical DMA bytes.

    ah/al_shard [n_rows_per_core, d] f16   (rows = contraction dim)
    abh/abl     [128, n_chunks*nB]   f16   (pre-swizzled, see _run_z)
    z_partial   [nB, d]              f32

    repeat > 1 wraps the body in a hardware loop; only used by the test
    harness to measure per-iteration device time by differencing.
    """
    import concourse.mybir as mybir
    import concourse.tile as tile
    from concourse import bacc

    P = 128
    assert n_rows_per_core % P == 0
    n_chunks = n_rows_per_core // P           # 8 for 1024 rows/core
    NT = 512                                  # one PSUM bank of f32 out
    assert d % NT == 0
    n_ntiles = d // NT                        # 2 for d=1024

    nc = bacc.Bacc("TRN2", target_bir_lowering=False, debug=False,
                   num_devices=n_cores)
    ah_in = nc.dram_tensor("ah_shard", [n_rows_per_core, d],
                           mybir.dt.float16, kind="ExternalInput")
    al_in = nc.dram_tensor("al_shard", [n_rows_per_core, d],
                           mybir.dt.float16, kind="ExternalInput")
    abh_in = nc.dram_tensor("abh_shard", [P, n_chunks * nB],
                            mybir.dt.float16, kind="ExternalInput")
    abl_in = nc.dram_tensor("abl_shard", [P, n_chunks * nB],
                            mybir.dt.float16, kind="ExternalInput")
    z_out = nc.dram_tensor("z_partial", [nB, d],
                           mybir.dt.float32, kind="ExternalOutput")

    # row t*128+p lives at partition p
    ah_view = ah_in.rearrange("(t p) m -> p t m", p=P)
    al_view = al_in.rearrange("(t p) m -> p t m", p=P)

    with tile.TileContext(nc) as tc:
        with tc.tile_pool(name="achunk", bufs=2 * n_chunks) as apool, \
             tc.tile_pool(name="ab", bufs=2) as abpool, \
             tc.tile_pool(name="zout", bufs=n_ntiles) as zpool, \
             tc.tile_pool(name="psum", bufs=1, space="PSUM") as psum:

            def body(_i=None):
                # small stationary operands on the ACT HWDGE ring so they
                # don't delay the A streams
                abh_sb = abpool.tile([P, n_chunks * nB], mybir.dt.float16,
                                     name="abh_sb", tag="ab")
                nc.scalar.dma_start(abh_sb[:], abh_in[:])
                abl_sb = abpool.tile([P, n_chunks * nB], mybir.dt.float16,
                                     name="abl_sb", tag="ab")
                nc.scalar.dma_start(abl_sb[:], abl_in[:])

                pts = [psum.tile([nB, NT], mybir.dt.float32, name=f"pt{h}",
                                 tag=f"pt{h}")
                       for h in range(n_ntiles)]
                ah_tiles, al_tiles = [], []
                for t in range(n_chunks):
                    ah_sb = apool.tile([P, d], mybir.dt.float16,
                                       name="ah_sb", tag="achunk")
                    al_sb = apool.tile([P, d], mybir.dt.float16,
                                       name="al_sb", tag="achunk")
                    if t == 0:
                        # split the first chunk so the PE can start sooner
                        for h in range(n_ntiles):
                            nc.sync.dma_start(
                                ah_sb[:, h * NT:(h + 1) * NT],
                                ah_view[:, t, h * NT:(h + 1) * NT])
                    else:
                        nc.sync.dma_start(ah_sb[:], ah_view[:, t, :])
                    # lo stream on the ACT ring (hi on SP) to halve the
                    # per-ring issue serialization
                    nc.scalar.dma_start(al_sb[:], al_view[:, t, :])
                    ah_tiles.append(ah_sb)
                    al_tiles.append(al_sb)
                n_terms = 3
                for t in range(n_chunks):
                    hiT = abh_sb[:, t * nB:(t + 1) * nB]
                    loT = abl_sb[:, t * nB:(t + 1) * nB]
                    # group by stationary operand to minimize LDWEIGHTS
                    for h in range(n_ntiles):
                        ah_h = ah_tiles[t][:, h * NT:(h + 1) * NT]
                        al_h = al_tiles[t][:, h * NT:(h + 1) * NT]
                        nc.tensor.matmul(pts[h][:], hiT, ah_h,
                                         start=(t == 0), stop=False)
                        nc.tensor.matmul(pts[h][:], hiT, al_h,
                                         start=False, stop=False)
                    for h in range(n_ntiles):
                        ah_h = ah_tiles[t][:, h * NT:(h + 1) * NT]
                        nc.tensor.matmul(
                            pts[h][:], loT, ah_h, start=False,
                            stop=(t == n_chunks - 1))
                # pt[0]'s last matmul lands before pt[1]'s, so its
                # PSUM->SBUF copy overlaps the final matmuls.
                for h in range(n_ntiles):
                    z_sb = zpool.tile([nB, NT], mybir.dt.float32,
                                      name="z_sb", tag="zout")
                    nc.vector.tensor_copy(z_sb[:], pts[h][:])
                    nc.sync.dma_start(z_out[:, h * NT:(h + 1) * NT],
                                      z_sb[:])

            if repeat == 1:
                body()
            else:
                with tc.For_i(0, repeat, 1) as i:
                    body(i)
    nc.compile()
    return nc


def _run_z(A, AB, n_cores=8):
    """Compute Z = AB^T @ A on the 8 NeuronCores (row-sharded)."""
    from concourse.bass_utils import run_bass_kernel_spmd

    n, d = A.shape
    nB = AB.shape[1]
    rows_per_core = n // n_cores
    key = (rows_per_core, d, nB, n_cores)
    if key not in _CACHE:
        _CACHE[key] = _build_z_kernel(rows_per_core, d, nB, n_cores)
    nc = _CACHE[key]

    # f16 hi/lo split (lo = A - hi is exact in f32)
    Ah = A.astype(np.float16)
    Al = (A - Ah.astype(np.float32)).astype(np.float16)
    # pre-swizzle AB into the kernel's SBUF layout:
    # [n, nB] -> per core [128, n_chunks*nB] with ab[p, t*nB+b] = AB[t*128+p, b]
    n_chunks = rows_per_core // 128

    def swizzle(X):
        return np.ascontiguousarray(
            X.reshape(n_cores, n_chunks, 128, nB)
            .transpose(0, 2, 1, 3)
            .reshape(n_cores, 128, n_chunks * nB))

    ABh = AB.astype(np.float16)
    ABl = (AB - ABh.astype(np.float32)).astype(np.float16)
    ABh_sw = swizzle(ABh)
    ABl_sw = swizzle(ABl)
    in_maps = []
    for c in range(n_cores):
        sl = slice(c * rows_per_core, (c + 1) * rows_per_core)
        in_maps.append({
            "ah_shard": np.ascontiguousarray(Ah[sl]),
            "al_shard": np.ascontiguousarray(Al[sl]),
            "abh_shard": ABh_sw[c],
            "abl_shard": ABl_sw[c],
        })
    res = run_bass_kernel_spmd(nc, in_maps, list(range(n_cores)))
    parts = np.stack([res.results[c]["z_partial"] for c in range(n_cores)])
    return parts.astype(np.float64).sum(axis=0)


# ------------------------------------------------------------------- host ---

def _host_reference_bits(A, S, num_samples):
    """The pieces that must match the reference bit-for-bit: f32 column
    norms (the 1e-5 match threshold has ~1e-6 margins) and the RNG draws
    (input-independent, key 42)."""
    import jax
    import jax.numpy as jnp

    cpu = jax.devices("cpu")[0]
    with jax.default_device(cpu):
        a_norms = np.asarray(jnp.linalg.norm(jnp.asarray(A), axis=0))
        s_norms = np.asarray(jnp.linalg.norm(jnp.asarray(S), axis=0))
        kg, km = jax.random.split(jax.random.key(42))
        u = np.asarray(jax.random.uniform(kg, (A.shape[1],),
                                          dtype=jnp.float32))
        rand_idx = int(np.asarray(
            jax.random.randint(km, (), 0, num_samples)))
    return a_norms, s_norms, u, rand_idx


def _topk_desc_stable(values, k):
    """jax.lax.top_k semantics: k largest, ties -> lower index first."""
    order = np.argsort(-values, kind="stable")
    return order[:k]


def _pinv_jaxlike(mats):
    """Batched pseudo-inverse with jax's f32 pinv rank cutoff
    (rtol = max(M,N) * eps_f32 relative to the largest singular value)."""
    u, s, vh = np.linalg.svd(mats)
    cutoff = (mats.shape[-1] * np.finfo(np.float32).eps
              * s[..., :1])
    s_inv = np.where(s > cutoff, 1.0 / np.where(s > 0, s, 1.0), 0.0)
    return np.einsum("...ji,...j,...kj->...ik", vh, s_inv, u)


def kernel(A_prime, k, S):
    A = np.ascontiguousarray(np.asarray(A_prime, dtype=np.float32))
    S = np.ascontiguousarray(np.asarray(S, dtype=np.float32))
    kk = int(np.asarray(k))
    n, d = A.shape
    s = S.shape[1]
    num_samples = min(10 * kk, d)

    a_norms, s_norms, u, rand_idx = _host_reference_bits(A, S, num_samples)

    # I_soft: columns of A matching a column of S by relative norm
    a64 = a_norms.astype(np.float64)
    s64 = s_norms.astype(np.float64)
    match = (np.abs(s64[None, :] - a64[:, None])
             / (a64[:, None] + EPS)) < 1e-5
    I_soft = match.any(axis=1).astype(np.float32)
    sel_idx = np.sort(_topk_desc_stable(I_soft, s))

    # G_S and the projection weights (small, host f64; margins ~7e-3)
    S64 = S.astype(np.float64)
    G_S = S64.T @ S64
    T = S64.T @ A.astype(np.float64)                  # [s, d]
    W = np.linalg.pinv(G_S) @ T
    a2 = a64 * a64
    col_norms = np.maximum(a2 - np.einsum("sd,sd->d", T, W), 0.0)

    probs = col_norms / (col_norms.sum() + EPS)
    gumbel = -np.log(-np.log(u.astype(np.float64) + EPS) + EPS)
    logits = np.log(probs + EPS) + gumbel
    C_indices = _topk_desc_stable(logits, num_samples)

    # --- device: Z = A[:, B]^T A, row-sharded over the 8 cores ---
    B = np.concatenate([sel_idx, C_indices]).astype(np.int64)
    AB = np.ascontiguousarray(A[:, B])
    Z = _run_z(A, AB)                                  # [s+ns, d] float64

    Ksub = Z[:, B]                                     # K[B, B]
    K2sub = Z @ Z.T                                    # K2[B, B]
    A_fro2 = float(a2.sum())

    # --- 640 pair objectives (tiny, host f64) ---
    ns = num_samples
    sel_pos = np.arange(s)
    # G/M for each candidate p: rows/cols [0..s-1] = sel, row/col s = p
    idx9 = np.empty((ns, s + 1), np.int64)
    idx9[:, :s] = np.arange(s)[None, :]
    idx9[:, s] = s + np.arange(ns)
    Gall = Ksub[idx9[:, :, None], idx9[:, None, :]]    # [ns, 9, 9]
    Mall = K2sub[idx9[:, :, None], idx9[:, None, :]]
    # masks: [ns, s, 9]: remove qpos; if p == sel[q], remove p too
    mask = np.ones((ns, s, s + 1))
    mask[:, sel_pos, sel_pos] = 0.0
    p_eq_q = (C_indices[:, None] == sel_idx[None, :])  # [ns, s]
    mask[:, :, s] = np.where(p_eq_q, 0.0, 1.0)
    mm = mask[:, :, :, None] * mask[:, :, None, :]     # [ns, s, 9, 9]
    Gm = mm * Gall[:, None]
    Mm = mm * Mall[:, None]
    pinvs = _pinv_jaxlike(Gm.reshape(-1, s + 1, s + 1))
    tr = np.einsum("bij,bij->b", pinvs,
                   Mm.reshape(-1, s + 1, s + 1))
    objs = np.sqrt(np.maximum(A_fro2 - tr, 0.0)).reshape(ns, s)

    amin = int(np.argmin(objs.reshape(-1)))
    min_idx = int(sel_idx[amin % s])
    best_p_idx = int(C_indices[rand_idx])

    I_final = I_soft.copy()
    I_final[min_idx] = 0.0
    I_final[best_p_idx] = 1.0
    out_idx = np.sort(_topk_desc_stable(I_final, s))
    return np.ascontiguousarray(A[:, out_idx])
